# revision 2
# baseline (speedup 1.0000x reference)
"""Winding-number field (differentiable voxelizer) on 8 Trainium2 NeuronCores.

v2 strategy — minimize instruction count (this backend charges ~10-100us
PER INSTRUCTION regardless of size, engines overlap):

  Layout: verts on partitions (64 tiles of 128), ALL core points (4096) on
  the free dim. Per vert-tile, the whole P-slice is processed by 9 big ops:

    t1 = (-2*vx)[part] * px[free] + pp[free]          VectorE STT
    t2 = (-2*vy)[part] * py[free] + t1                VectorE STT
    t3 = (-2*vz)[part] * pz[free] + t2                VectorE STT
    u  = Ln(t3 + (vv + B_REG)[part])                  ScalarE act (bias AP)
    s  = Exp(-1.5 * u)       ( = (r2+b)^-1.5 )        ScalarE act (in-place)
    partial_c += w_c[part] * s   (c in x,y,z,d)       VectorE STT in-place

  Then combined = partial_d - px*partial_x - py*partial_y - pz*partial_z
  (6 VectorE ops), cross-partition sum via 8 ones-vector matmuls (PE), DMA
  straight from PSUM. 8 matmuls total vs 1024 in the v1 kernel.

  Host: bit-exact areaic normals (fp32 jax CPU) + near-pair correction for
  r < RCUT (device value predicted in fp64, B_REG floor makes fp32
  rounding-order differences negligible) — unchanged from v1.
"""

import os
import sys

import numpy as np

for _p in ("/opt/trn_rl_repo", "/root/.axon_site/_ro/trn_rl_repo"):
    if _p not in sys.path and os.path.isdir(_p):
        sys.path.insert(0, _p)

from contextlib import ExitStack

import concourse.bass as bass  # noqa: E402
import concourse.tile as tile  # noqa: E402
from concourse import bacc, mybir  # noqa: E402
from concourse.bass import ds  # noqa: E402
from concourse.bass_utils import run_bass_kernel_spmd  # noqa: E402

EPS = 1e-8          # reference epsilon in 1/(r^3 + EPS)
B_REG = 1e-4        # device regularizer: s = (r2 + B_REG)^-1.5
RCUT = 0.3          # host-corrected pair radius
FOUR_PI = 4.0 * np.pi

N_CORES = 8
V = 8192
P = 32768
PC = P // N_CORES         # 4096 points per core = free dim
VT = 128                  # vert tile (partition dim)
NVT = V // VT             # 64 vert tiles
FD = PC
F32 = mybir.dt.float32
ALU = mybir.AluOpType

_NC_CACHE = {}


class _OneSetBacc(bacc.Bacc):
    """Bacc whose activation-table pass only sees `natural_log_exp_and_others`
    (contains ln and exp) so a single ACT_TABLE_LOAD is hoisted."""

    def insert_act_table_loads(self):
        import bass_rust as _bass_rust
        from concourse.hw_specs import get_activation_tables

        has_activation = any(
            isinstance(i, mybir.InstActivation)
            for b in self.main_func.blocks
            for i in b.instructions
        )
        if not has_activation:
            return
        keep = {"natural_log_exp_and_others"}
        tables = [(k, v if k in keep else set())
                  for k, v in get_activation_tables(self.m.arch).items()]
        assert any(v for _, v in tables), "required activation sets missing"
        _bass_rust.insert_act_table_loads(self, tables)


UNROLL2 = False
COMB_LOOP = False
SPLITP = False


def _build_nc(reps=1, skip_chan=False, skip_act=False, skip_r2=False,
              unroll2=UNROLL2, comb_loop=COMB_LOOP, splitp=SPLITP,
              staggered=False, hints=False, pipe_mode=False, pipe_unroll=2):
    nc = _OneSetBacc("TRN2", target_bir_lowering=False, debug=False)

    px_d = nc.declare_dram_parameter("px", [1, FD], F32, isOutput=False)
    py_d = nc.declare_dram_parameter("py", [1, FD], F32, isOutput=False)
    pz_d = nc.declare_dram_parameter("pz", [1, FD], F32, isOutput=False)
    pp_d = nc.declare_dram_parameter("pp", [1, FD], F32, isOutput=False)
    # cols[part, k*NVT + vt]: k = 0..7 -> m2x, m2y, m2z, vvb, wx, wy, wz, wd
    cols_d = nc.declare_dram_parameter("cols", [VT, 8 * NVT], F32,
                                       isOutput=False)
    out_d = nc.declare_dram_parameter("outw", [1, FD], F32, isOutput=True)

    with ExitStack() as ctx:
        tc = ctx.enter_context(tile.TileContext(nc))
        consts = ctx.enter_context(tc.tile_pool(name="consts", bufs=1))
        psum = ctx.enter_context(tc.tile_pool(name="psum", bufs=1,
                                              space="PSUM"))

        if splitp:
            comb_loop = False
            pxt = consts.tile([VT, FD], F32)
            pyt = consts.tile([VT, FD], F32)
            pzt = consts.tile([VT, FD], F32)
            pxr, pyr, pzr = pxt[:], pyt[:], pzt[:]
            prs = None
        else:
            prs = consts.tile([VT, 3 * FD], F32)   # px | py | pz replicated
            pxr = prs[:, 0 * FD:1 * FD]
            pyr = prs[:, 1 * FD:2 * FD]
            pzr = prs[:, 2 * FD:3 * FD]
        ppr = consts.tile([VT, FD], F32)
        cols = consts.tile([VT, 8 * NVT], F32)
        partial = consts.tile([VT, 4 * FD], F32)   # 64 KB/partition
        if not pipe_mode:
            tA = consts.tile([VT, FD], F32)
            uA = consts.tile([VT, FD], F32)
        if unroll2:
            tB = consts.tile([VT, FD], F32)
            uB = consts.tile([VT, FD], F32)
        ones = consts.tile([VT, 1], F32)
        ps = psum.tile([1, 512], F32, tag="ps")
        nc.vector.memset(ones[:], 1.0)
        nc.sync.dma_start(out=pxr, in_=px_d.ap().broadcast_to([VT, FD]))
        nc.sync.dma_start(out=pyr, in_=py_d.ap().broadcast_to([VT, FD]))
        nc.sync.dma_start(out=pzr, in_=pz_d.ap().broadcast_to([VT, FD]))
        nc.sync.dma_start(out=ppr[:], in_=pp_d.ap().broadcast_to([VT, FD]))
        nc.sync.dma_start(out=cols[:], in_=cols_d.ap())

        def pch(c):
            return partial[:, c * FD:(c + 1) * FD]

        def r2_chain(t, vt):
            """t = r2 + b for vert tile vt (4 VectorE ops, fp32).

            Activation bias APs can't take symbolic offsets on this
            backend, so r2+b is finished on the vector engine and Ln
            runs bias-free."""
            if skip_r2:
                nc.vector.memset(t[:], 1.0)
                return
            # t = (-2vx)*px + (vv+b)   [two per-partition scalars]
            nc.vector.tensor_scalar(
                t[:], pxr, cols[:, ds(vt, 1)],
                cols[:, ds(vt + 3 * NVT, 1)],
                ALU.mult, ALU.add)
            nc.vector.scalar_tensor_tensor(
                t[:], pyr, cols[:, ds(vt + NVT, 1)], t[:],
                op0=ALU.mult, op1=ALU.add)
            nc.vector.scalar_tensor_tensor(
                t[:], pzr, cols[:, ds(vt + 2 * NVT, 1)], t[:],
                op0=ALU.mult, op1=ALU.add)
            nc.vector.tensor_tensor(t[:], t[:], ppr[:], op=ALU.add)

        def s_acts(u, t):
            """u = (r2+b)^-1.5 (2 ScalarE activations)."""
            if skip_act:
                nc.scalar.copy(u[:], t[:])
                return
            nc.scalar.activation(u[:], t[:],
                                 mybir.ActivationFunctionType.Ln)
            nc.scalar.activation(u[:], u[:],
                                 mybir.ActivationFunctionType.Exp,
                                 scale=-1.5)

        def chan_acc(u, vt):
            """partial_c += w_c * s (4 VectorE ops)."""
            if skip_chan:
                return
            for c in range(4):
                nc.vector.scalar_tensor_tensor(
                    pch(c), u[:], cols[:, ds(vt + (4 + c) * NVT, 1)],
                    pch(c), op0=ALU.mult, op1=ALU.add)

        for rep in range(reps):
            nc.vector.memset(partial[:], 0.0)
            # hardware loop, 2 vert tiles per iteration with A/B tile pairs:
            # the ~20-instruction body is fetched once and re-executed
            # NVT/2 times (the dominant cost on this backend is
            # per-instruction-word fetch, ~60-90us).  The A/B split lets
            # ScalarE's Ln/Exp run under the vector engine's 8-op stretch.
            lkw = {}
            if staggered:
                lkw["staggered_reset"] = True
            if hints:
                lkw["hint_engines"] = (mybir.EngineType.DVE,
                                       mybir.EngineType.Activation)
            if pipe_mode:
                def stage_r2(pipe, iv):
                    t = pipe.intermediate_tile([VT, FD], F32)
                    nc.vector.tensor_scalar(
                        t[:], pxr, cols[:, ds(iv, 1)],
                        cols[:, ds(iv + 3 * NVT, 1)], ALU.mult, ALU.add)
                    nc.vector.scalar_tensor_tensor(
                        t[:], pyr, cols[:, ds(iv + NVT, 1)], t[:],
                        op0=ALU.mult, op1=ALU.add)
                    nc.vector.scalar_tensor_tensor(
                        t[:], pzr, cols[:, ds(iv + 2 * NVT, 1)], t[:],
                        op0=ALU.mult, op1=ALU.add)
                    nc.vector.tensor_tensor(t[:], t[:], ppr[:], op=ALU.add)
                    return t

                def stage_act(pipe, iv, t):
                    u = pipe.intermediate_tile([VT, FD], F32)
                    nc.scalar.activation(u[:], t[:],
                                         mybir.ActivationFunctionType.Ln)
                    nc.scalar.activation(u[:], u[:],
                                         mybir.ActivationFunctionType.Exp,
                                         scale=-1.5)
                    return u

                def stage_chan(pipe, iv, u):
                    for c in range(4):
                        nc.vector.scalar_tensor_tensor(
                            pch(c), u[:], cols[:, ds(iv + (4 + c) * NVT, 1)],
                            pch(c), op0=ALU.mult, op1=ALU.add)

                tc.For_i_pipelined([stage_r2, stage_act, stage_chan],
                                   0, NVT, unroll=pipe_unroll, **lkw)
            elif unroll2:
                with tc.For_i(0, NVT // 2, 1, **lkw) as i:
                    r2_chain(tA, i * 2)
                    s_acts(uA, tA)
                    r2_chain(tB, i * 2 + 1)
                    s_acts(uB, tB)
                    chan_acc(uA, i * 2)
                    chan_acc(uB, i * 2 + 1)
            else:
                with tc.For_i(0, NVT, 1, **lkw) as i:
                    r2_chain(tA, i)
                    s_acts(uA, tA)
                    chan_acc(uA, i)
            # combined (into ch 3): wd_sum - px*wx_sum - py*wy_sum - pz*wz_sum
            # (in-place on the dead channels 0..2: no scratch tile needed)
            if comb_loop:
                with tc.For_i(0, 3, 1) as c:
                    nc.vector.tensor_tensor(
                        partial[:, ds(c * FD, FD)], prs[:, ds(c * FD, FD)],
                        partial[:, ds(c * FD, FD)], op=ALU.mult)
                    nc.vector.tensor_tensor(pch(3), pch(3),
                                            partial[:, ds(c * FD, FD)],
                                            op=ALU.subtract)
            else:
                for c, pr in enumerate((pxr, pyr, pzr)):
                    nc.vector.tensor_tensor(pch(c), pr, pch(c), op=ALU.mult)
                    nc.vector.tensor_tensor(pch(3), pch(3), pch(c),
                                            op=ALU.subtract)
            # cross-partition reduce on PE (partial ch0 row 0 is dead:
            # use it as the DMA staging row)
            with tc.For_i(0, FD // 512, 1) as b:
                nc.tensor.matmul(ps[:], ones[:, 0:1],
                                 partial[:, ds(b * 512 + 3 * FD, 512)],
                                 start=True, stop=True)
                nc.vector.tensor_copy(partial[0:1, ds(b * 512, 512)], ps[:])
        nc.sync.dma_start(out=out_d.ap(), in_=partial[0:1, 0:FD])
    nc.finalize()
    return nc


# ------------------------- host-side numerics --------------------------------
def _preprocess_mesh(verts, faces):
    """Bit-exact replica of the reference's areaic normals: jax fp32 on CPU."""
    import jax
    import jax.numpy as jnp

    with jax.default_device(jax.devices("cpu")[0]):
        v = jnp.asarray(verts, jnp.float32)
        f = jnp.asarray(np.asarray(faces).astype(np.int32))
        fv = v[f]
        A = fv[:, 1] - fv[:, 0]
        Bv = fv[:, 2] - fv[:, 1]
        C = fv[:, 0] - fv[:, 2]

        def corner_angle(u, w):
            c = -jnp.sum(u * w, axis=1) / (
                EPS + jnp.linalg.norm(u, axis=1) * jnp.linalg.norm(w, axis=1))
            return jnp.arccos(jnp.clip(c, -1.0, 1.0))

        angles = jnp.stack(
            [corner_angle(C, A), corner_angle(A, Bv), corner_angle(Bv, C)],
            axis=1)
        s2 = jnp.sin(2.0 * angles)
        w = s2 / (jnp.sum(s2, axis=-1, keepdims=True) + EPS)
        w = (w[:, [2, 0, 1]] + w[:, [1, 2, 0]]) / 2.0

        fn = jnp.cross(A, Bv)
        areas = 0.5 * jnp.linalg.norm(fn, axis=1)

        nv = v.shape[0]
        idx = f.reshape(-1)
        dual_v = jax.ops.segment_sum((w * areas[:, None]).reshape(-1), idx,
                                     num_segments=nv)
        vn = jax.ops.segment_sum(jnp.repeat(fn, 3, axis=0), idx,
                                 num_segments=nv)
        vn = vn / (jnp.linalg.norm(vn, axis=1, keepdims=True) + EPS)
        na = np.asarray(vn * dual_v[:, None])
    d = np.sum(na.astype(np.float64) * np.asarray(verts, np.float64), axis=1)
    return na, d.astype(np.float32)


def _near_pairs(points, verts, rcut):
    """(point, vert) pairs with |p-v| < rcut via grid hashing (pure numpy)."""
    from collections import defaultdict

    pts = points.astype(np.float64)
    vts = verts.astype(np.float64)
    vcell = np.floor(vts / rcut).astype(np.int64)
    vmap = defaultdict(list)
    for j, c in enumerate(map(tuple, vcell)):
        vmap[c].append(j)
    vmap = {k: np.asarray(vs) for k, vs in vmap.items()}
    pcell = np.floor(pts / rcut).astype(np.int64)
    order = np.lexsort((pcell[:, 2], pcell[:, 1], pcell[:, 0]))
    pc_sorted = pcell[order]
    bounds = np.nonzero(np.any(np.diff(pc_sorted, axis=0) != 0, axis=1))[0] + 1
    starts = np.concatenate([[0], bounds])
    ends = np.concatenate([bounds, [len(order)]])
    ip_list, iv_list = [], []
    for s0, e0 in zip(starts, ends):
        pidx = order[s0:e0]
        c = pc_sorted[s0]
        cand = [vmap[k] for k in
                ((c[0] + dx, c[1] + dy, c[2] + dz)
                 for dx in (-1, 0, 1) for dy in (-1, 0, 1) for dz in (-1, 0, 1))
                if k in vmap]
        if not cand:
            continue
        cand = np.concatenate(cand)
        diff = vts[None, cand, :] - pts[pidx, None, :]
        r2 = np.sum(diff * diff, axis=2)
        ii, jj = np.nonzero(r2 < rcut * rcut)
        ip_list.append(pidx[ii])
        iv_list.append(cand[jj])
    if not ip_list:
        return np.zeros(0, np.int64), np.zeros(0, np.int64)
    return np.concatenate(ip_list), np.concatenate(iv_list)


def _host_correction(points32, verts32, na, pp32, vvb32):
    """wf_corr[p] = sum_near [s_true - s_devpred] * (na_v . (v-p)) / 4pi."""
    ip, iv = _near_pairs(points32, verts32, RCUT)
    p = points32.astype(np.float64)[ip]
    v = verts32.astype(np.float64)[iv]
    diff = v - p
    r2t = np.sum(diff * diff, axis=1)
    s_true = 1.0 / (r2t ** 1.5 + EPS)
    # predicted device value: r2+b = vvb + pp - 2 v.p (device fp32 rounding
    # differs ~1e-7 abs; the B_REG floor makes that negligible)
    r2b = vvb32.astype(np.float64)[iv] + pp32.astype(np.float64)[ip] \
        - 2.0 * np.sum(p * v, axis=1)
    r2b = np.maximum(r2b, B_REG * 0.5)
    s_dev = r2b ** -1.5
    g = np.sum(na.astype(np.float64)[iv] * diff, axis=1)
    corr = (s_true - s_dev) * g / FOUR_PI
    return np.bincount(ip, weights=corr, minlength=points32.shape[0])


# ------------------------------- entry point ---------------------------------
def _prepare(verts, points, faces):
    verts32 = np.ascontiguousarray(np.asarray(verts, np.float32))
    points32 = np.ascontiguousarray(np.asarray(points, np.float32))

    na, d = _preprocess_mesh(verts32, faces)

    vv32 = np.sum(verts32.astype(np.float64) ** 2, axis=1).astype(np.float32)
    pp32 = np.sum(points32.astype(np.float64) ** 2, axis=1).astype(np.float32)
    vvb32 = (vv32.astype(np.float64) + B_REG).astype(np.float32)

    # cols[part, k*NVT + vt] for vert v = vt*128 + part
    def vcol(a):  # (V,) -> (VT, NVT)
        return np.ascontiguousarray(
            a.reshape(NVT, VT).T.astype(np.float32))

    cols = np.empty((VT, 8 * NVT), np.float32)
    cols[:, 0 * NVT:1 * NVT] = vcol(-2.0 * verts32[:, 0])
    cols[:, 1 * NVT:2 * NVT] = vcol(-2.0 * verts32[:, 1])
    cols[:, 2 * NVT:3 * NVT] = vcol(-2.0 * verts32[:, 2])
    cols[:, 3 * NVT:4 * NVT] = vcol(vvb32)
    cols[:, 4 * NVT:5 * NVT] = vcol(na[:, 0].astype(np.float32))
    cols[:, 5 * NVT:6 * NVT] = vcol(na[:, 1].astype(np.float32))
    cols[:, 6 * NVT:7 * NVT] = vcol(na[:, 2].astype(np.float32))
    cols[:, 7 * NVT:8 * NVT] = vcol(d)

    in_maps = []
    for c in range(N_CORES):
        sl = slice(c * PC, (c + 1) * PC)
        in_maps.append({
            "px": np.ascontiguousarray(points32[sl, 0]).reshape(1, FD),
            "py": np.ascontiguousarray(points32[sl, 1]).reshape(1, FD),
            "pz": np.ascontiguousarray(points32[sl, 2]).reshape(1, FD),
            "pp": np.ascontiguousarray(pp32[sl]).reshape(1, FD),
            "cols": cols,
        })
    return in_maps, verts32, points32, na, pp32, vvb32


def _finish(core_outs, verts32, points32, na, pp32, vvb32):
    """core_outs: list of (1, FD) combined fields. + near-pair correction."""
    wf = np.empty(P, np.float64)
    for c in range(N_CORES):
        sl = slice(c * PC, (c + 1) * PC)
        wf[sl] = np.asarray(core_outs[c], np.float64)[0] / FOUR_PI
    wf += _host_correction(points32, verts32, na, pp32, vvb32)
    return wf.astype(np.float32)


def kernel(verts, points, faces):
    import time

    in_maps, verts32, points32, na, pp32, vvb32 = _prepare(verts, points,
                                                           faces)
    last_err = None
    for attempt in range(3):
        try:
            if "nc" not in _NC_CACHE:
                _NC_CACHE["nc"] = _build_nc()
            res = run_bass_kernel_spmd(_NC_CACHE["nc"], in_maps,
                                       list(range(N_CORES)))
            core_outs = [np.asarray(res.results[c]["outw"])
                         for c in range(N_CORES)]
            break
        except Exception as e:  # transient axon/NRT faults: rebuild + retry
            last_err = e
            _NC_CACHE.clear()
            time.sleep(5 * (attempt + 1))
    else:
        raise last_err
    return _finish(core_outs, verts32, points32, na, pp32, vvb32)


# revision 3
# speedup vs baseline: 2.8376x; 2.8376x over previous
"""Winding-number field (differentiable voxelizer) on 8 Trainium2 NeuronCores.

v2 strategy — minimize instruction count (this backend charges ~10-100us
PER INSTRUCTION regardless of size, engines overlap):

  Layout: verts on partitions (64 tiles of 128), ALL core points (4096) on
  the free dim. Per vert-tile, the whole P-slice is processed by 9 big ops:

    t1 = (-2*vx)[part] * px[free] + pp[free]          VectorE STT
    t2 = (-2*vy)[part] * py[free] + t1                VectorE STT
    t3 = (-2*vz)[part] * pz[free] + t2                VectorE STT
    u  = Ln(t3 + (vv + B_REG)[part])                  ScalarE act (bias AP)
    s  = Exp(-1.5 * u)       ( = (r2+b)^-1.5 )        ScalarE act (in-place)
    partial_c += w_c[part] * s   (c in x,y,z,d)       VectorE STT in-place

  Then combined = partial_d - px*partial_x - py*partial_y - pz*partial_z
  (6 VectorE ops), cross-partition sum via 8 ones-vector matmuls (PE), DMA
  straight from PSUM. 8 matmuls total vs 1024 in the v1 kernel.

  Host: bit-exact areaic normals (fp32 jax CPU) + near-pair correction for
  r < RCUT (device value predicted in fp64, B_REG floor makes fp32
  rounding-order differences negligible) — unchanged from v1.
"""

import os
import sys

import numpy as np

for _p in ("/opt/trn_rl_repo", "/root/.axon_site/_ro/trn_rl_repo"):
    if _p not in sys.path and os.path.isdir(_p):
        sys.path.insert(0, _p)

from contextlib import ExitStack

import concourse.bass as bass  # noqa: E402
import concourse.tile as tile  # noqa: E402
from concourse import bacc, mybir  # noqa: E402
from concourse.bass import ds  # noqa: E402
from concourse.bass_utils import run_bass_kernel_spmd  # noqa: E402

EPS = 1e-8          # reference epsilon in 1/(r^3 + EPS)
B_REG = 1e-4        # device regularizer: s = (r2 + B_REG)^-1.5
RCUT = 0.3          # host-corrected pair radius
FOUR_PI = 4.0 * np.pi

N_CORES = 8
V = 8192
P = 32768
PC = P // N_CORES         # 4096 points per core = free dim
VT = 128                  # vert tile (partition dim)
NVT = V // VT             # 64 vert tiles
FD = PC
F32 = mybir.dt.float32
ALU = mybir.AluOpType

_NC_CACHE = {}


class _OneSetBacc(bacc.Bacc):
    """Bacc whose activation-table pass only sees `natural_log_exp_and_others`
    (contains ln and exp) so a single ACT_TABLE_LOAD is hoisted."""

    def insert_act_table_loads(self):
        import bass_rust as _bass_rust
        from concourse.hw_specs import get_activation_tables

        has_activation = any(
            isinstance(i, mybir.InstActivation)
            for b in self.main_func.blocks
            for i in b.instructions
        )
        if not has_activation:
            return
        keep = {"natural_log_exp_and_others"}
        tables = [(k, v if k in keep else set())
                  for k, v in get_activation_tables(self.m.arch).items()]
        assert any(v for _, v in tables), "required activation sets missing"
        _bass_rust.insert_act_table_loads(self, tables)


UNROLL2 = False
COMB_LOOP = False
SPLITP = False


def _build_nc(reps=1, skip_chan=False, skip_act=False, skip_r2=False,
              unroll2=UNROLL2, comb_loop=COMB_LOOP, splitp=SPLITP,
              staggered=False, hints=False, pipe_mode=False, pipe_unroll=2):
    nc = _OneSetBacc("TRN2", target_bir_lowering=False, debug=False)

    px_d = nc.declare_dram_parameter("px", [1, FD], F32, isOutput=False)
    py_d = nc.declare_dram_parameter("py", [1, FD], F32, isOutput=False)
    pz_d = nc.declare_dram_parameter("pz", [1, FD], F32, isOutput=False)
    pp_d = nc.declare_dram_parameter("pp", [1, FD], F32, isOutput=False)
    # cols[part, k*NVT + vt]: k = 0..7 -> m2x, m2y, m2z, vvb, wx, wy, wz, wd
    cols_d = nc.declare_dram_parameter("cols", [VT, 8 * NVT], F32,
                                       isOutput=False)
    out_d = nc.declare_dram_parameter("outw", [4, FD], F32, isOutput=True)

    with ExitStack() as ctx:
        tc = ctx.enter_context(tile.TileContext(nc))
        consts = ctx.enter_context(tc.tile_pool(name="consts", bufs=1))
        psum = ctx.enter_context(tc.tile_pool(name="psum", bufs=1,
                                              space="PSUM"))

        prs = consts.tile([VT, 3 * FD], F32)   # px | py | pz replicated
        pxr = prs[:, 0 * FD:1 * FD]
        pyr = prs[:, 1 * FD:2 * FD]
        pzr = prs[:, 2 * FD:3 * FD]
        ppr = consts.tile([VT, FD], F32)
        # cols: k*NVT+vt for k in (m2x, m2y, m2z, vvb), then n4[part, vt*4+c]
        cols = consts.tile([VT, 8 * NVT], F32)
        tA = consts.tile([VT, FD], F32)
        uA = consts.tile([VT, FD], F32)
        zeros4 = consts.tile([VT, 4], F32)
        n4s = consts.tile([VT, 4], F32)
        acc = psum.tile([4, FD], F32, tag="acc")   # all 8 PSUM banks

        nc.sync.dma_start(out=pxr, in_=px_d.ap().broadcast_to([VT, FD]))
        nc.sync.dma_start(out=pyr, in_=py_d.ap().broadcast_to([VT, FD]))
        nc.sync.dma_start(out=pzr, in_=pz_d.ap().broadcast_to([VT, FD]))
        nc.sync.dma_start(out=ppr[:], in_=pp_d.ap().broadcast_to([VT, FD]))
        nc.sync.dma_start(out=cols[:], in_=cols_d.ap())
        nc.vector.memset(zeros4[:], 0.0)

        for rep in range(reps):
            # zero the accumulation banks (start=True) with a 0-contribution
            for b in range(8):
                nc.tensor.matmul(acc[:, b * 512:(b + 1) * 512],
                                 zeros4[:, 0:4], prs[:, 0:512],
                                 start=True, stop=False)
            # hardware loop over vert tiles: 4 vector ops (r2+b), 2 acts
            # (s = (r2+b)^-1.5), 8 PE matmuls accumulating
            # acc[c, p] += n4[v, c] * s[v, p] into PSUM.
            lkw = {}
            if staggered:
                lkw["staggered_reset"] = True
            if hints:
                lkw["hint_engines"] = (mybir.EngineType.DVE,
                                       mybir.EngineType.Activation,
                                       mybir.EngineType.PE)
            with tc.For_i(0, NVT, 1, **lkw) as i:
                nc.vector.tensor_scalar(
                    tA[:], pxr, cols[:, ds(i, 1)],
                    cols[:, ds(i + 3 * NVT, 1)], ALU.mult, ALU.add)
                nc.vector.scalar_tensor_tensor(
                    tA[:], pyr, cols[:, ds(i + NVT, 1)], tA[:],
                    op0=ALU.mult, op1=ALU.add)
                nc.vector.scalar_tensor_tensor(
                    tA[:], pzr, cols[:, ds(i + 2 * NVT, 1)], tA[:],
                    op0=ALU.mult, op1=ALU.add)
                nc.vector.tensor_tensor(tA[:], tA[:], ppr[:], op=ALU.add)
                nc.scalar.activation(uA[:], tA[:],
                                     mybir.ActivationFunctionType.Ln)
                nc.scalar.activation(uA[:], uA[:],
                                     mybir.ActivationFunctionType.Exp,
                                     scale=-1.5)
                # ldweights can't take a register offset: stage this vert
                # tile's n4 columns into a fixed tile first
                nc.vector.tensor_copy(n4s[:], cols[:, ds(i * 4 + 4 * NVT, 4)])
                for b in range(8):
                    nc.tensor.matmul(acc[:, b * 512:(b + 1) * 512],
                                     n4s[:, 0:4],
                                     uA[:, b * 512:(b + 1) * 512],
                                     start=False, stop=False)
            # close the accumulation groups with another 0-contribution
            for b in range(8):
                nc.tensor.matmul(acc[:, b * 512:(b + 1) * 512],
                                 zeros4[:, 0:4], prs[:, 0:512],
                                 start=False, stop=True)
            nc.vector.tensor_copy(tA[0:4, 0:FD], acc[:])
        nc.sync.dma_start(out=out_d.ap(), in_=tA[0:4, 0:FD])
    nc.finalize()
    return nc


# ------------------------- host-side numerics --------------------------------
def _preprocess_mesh(verts, faces):
    """Bit-exact replica of the reference's areaic normals: jax fp32 on CPU."""
    import jax
    import jax.numpy as jnp

    with jax.default_device(jax.devices("cpu")[0]):
        v = jnp.asarray(verts, jnp.float32)
        f = jnp.asarray(np.asarray(faces).astype(np.int32))
        fv = v[f]
        A = fv[:, 1] - fv[:, 0]
        Bv = fv[:, 2] - fv[:, 1]
        C = fv[:, 0] - fv[:, 2]

        def corner_angle(u, w):
            c = -jnp.sum(u * w, axis=1) / (
                EPS + jnp.linalg.norm(u, axis=1) * jnp.linalg.norm(w, axis=1))
            return jnp.arccos(jnp.clip(c, -1.0, 1.0))

        angles = jnp.stack(
            [corner_angle(C, A), corner_angle(A, Bv), corner_angle(Bv, C)],
            axis=1)
        s2 = jnp.sin(2.0 * angles)
        w = s2 / (jnp.sum(s2, axis=-1, keepdims=True) + EPS)
        w = (w[:, [2, 0, 1]] + w[:, [1, 2, 0]]) / 2.0

        fn = jnp.cross(A, Bv)
        areas = 0.5 * jnp.linalg.norm(fn, axis=1)

        nv = v.shape[0]
        idx = f.reshape(-1)
        dual_v = jax.ops.segment_sum((w * areas[:, None]).reshape(-1), idx,
                                     num_segments=nv)
        vn = jax.ops.segment_sum(jnp.repeat(fn, 3, axis=0), idx,
                                 num_segments=nv)
        vn = vn / (jnp.linalg.norm(vn, axis=1, keepdims=True) + EPS)
        na = np.asarray(vn * dual_v[:, None])
    d = np.sum(na.astype(np.float64) * np.asarray(verts, np.float64), axis=1)
    return na, d.astype(np.float32)


def _near_pairs(points, verts, rcut):
    """(point, vert) pairs with |p-v| < rcut via grid hashing (pure numpy)."""
    from collections import defaultdict

    pts = points.astype(np.float64)
    vts = verts.astype(np.float64)
    vcell = np.floor(vts / rcut).astype(np.int64)
    vmap = defaultdict(list)
    for j, c in enumerate(map(tuple, vcell)):
        vmap[c].append(j)
    vmap = {k: np.asarray(vs) for k, vs in vmap.items()}
    pcell = np.floor(pts / rcut).astype(np.int64)
    order = np.lexsort((pcell[:, 2], pcell[:, 1], pcell[:, 0]))
    pc_sorted = pcell[order]
    bounds = np.nonzero(np.any(np.diff(pc_sorted, axis=0) != 0, axis=1))[0] + 1
    starts = np.concatenate([[0], bounds])
    ends = np.concatenate([bounds, [len(order)]])
    ip_list, iv_list = [], []
    for s0, e0 in zip(starts, ends):
        pidx = order[s0:e0]
        c = pc_sorted[s0]
        cand = [vmap[k] for k in
                ((c[0] + dx, c[1] + dy, c[2] + dz)
                 for dx in (-1, 0, 1) for dy in (-1, 0, 1) for dz in (-1, 0, 1))
                if k in vmap]
        if not cand:
            continue
        cand = np.concatenate(cand)
        diff = vts[None, cand, :] - pts[pidx, None, :]
        r2 = np.sum(diff * diff, axis=2)
        ii, jj = np.nonzero(r2 < rcut * rcut)
        ip_list.append(pidx[ii])
        iv_list.append(cand[jj])
    if not ip_list:
        return np.zeros(0, np.int64), np.zeros(0, np.int64)
    return np.concatenate(ip_list), np.concatenate(iv_list)


def _host_correction(points32, verts32, na, pp32, vvb32):
    """wf_corr[p] = sum_near [s_true - s_devpred] * (na_v . (v-p)) / 4pi."""
    ip, iv = _near_pairs(points32, verts32, RCUT)
    p = points32.astype(np.float64)[ip]
    v = verts32.astype(np.float64)[iv]
    diff = v - p
    r2t = np.sum(diff * diff, axis=1)
    s_true = 1.0 / (r2t ** 1.5 + EPS)
    # predicted device value: r2+b = vvb + pp - 2 v.p (device fp32 rounding
    # differs ~1e-7 abs; the B_REG floor makes that negligible)
    r2b = vvb32.astype(np.float64)[iv] + pp32.astype(np.float64)[ip] \
        - 2.0 * np.sum(p * v, axis=1)
    r2b = np.maximum(r2b, B_REG * 0.5)
    s_dev = r2b ** -1.5
    g = np.sum(na.astype(np.float64)[iv] * diff, axis=1)
    corr = (s_true - s_dev) * g / FOUR_PI
    return np.bincount(ip, weights=corr, minlength=points32.shape[0])


# ------------------------------- entry point ---------------------------------
def _prepare(verts, points, faces):
    verts32 = np.ascontiguousarray(np.asarray(verts, np.float32))
    points32 = np.ascontiguousarray(np.asarray(points, np.float32))

    na, d = _preprocess_mesh(verts32, faces)

    vv32 = np.sum(verts32.astype(np.float64) ** 2, axis=1).astype(np.float32)
    pp32 = np.sum(points32.astype(np.float64) ** 2, axis=1).astype(np.float32)
    vvb32 = (vv32.astype(np.float64) + B_REG).astype(np.float32)

    # cols[part, k*NVT + vt] for vert v = vt*128 + part
    def vcol(a):  # (V,) -> (VT, NVT)
        return np.ascontiguousarray(
            a.reshape(NVT, VT).T.astype(np.float32))

    cols = np.empty((VT, 8 * NVT), np.float32)
    cols[:, 0 * NVT:1 * NVT] = vcol(-2.0 * verts32[:, 0])
    cols[:, 1 * NVT:2 * NVT] = vcol(-2.0 * verts32[:, 1])
    cols[:, 2 * NVT:3 * NVT] = vcol(-2.0 * verts32[:, 2])
    cols[:, 3 * NVT:4 * NVT] = vcol(vvb32)
    nmat = np.concatenate([na.astype(np.float32), d[:, None]], axis=1)
    cols[:, 4 * NVT:8 * NVT] = np.ascontiguousarray(
        nmat.reshape(NVT, VT, 4).transpose(1, 0, 2).reshape(VT, NVT * 4))

    in_maps = []
    for c in range(N_CORES):
        sl = slice(c * PC, (c + 1) * PC)
        in_maps.append({
            "px": np.ascontiguousarray(points32[sl, 0]).reshape(1, FD),
            "py": np.ascontiguousarray(points32[sl, 1]).reshape(1, FD),
            "pz": np.ascontiguousarray(points32[sl, 2]).reshape(1, FD),
            "pp": np.ascontiguousarray(pp32[sl]).reshape(1, FD),
            "cols": cols,
        })
    return in_maps, verts32, points32, na, pp32, vvb32


def _finish(core_outs, verts32, points32, na, pp32, vvb32):
    """core_outs: list of (4, FD) [A_xyz | B] rows. Host combine +
    near-pair correction."""
    wf = np.empty(P, np.float64)
    for c in range(N_CORES):
        sl = slice(c * PC, (c + 1) * PC)
        o = np.asarray(core_outs[c], np.float64)
        pd = points32[sl].astype(np.float64)
        wf[sl] = (o[3] - pd[:, 0] * o[0] - pd[:, 1] * o[1]
                  - pd[:, 2] * o[2]) / FOUR_PI
    wf += _host_correction(points32, verts32, na, pp32, vvb32)
    return wf.astype(np.float32)


def kernel(verts, points, faces):
    import time

    in_maps, verts32, points32, na, pp32, vvb32 = _prepare(verts, points,
                                                           faces)
    last_err = None
    for attempt in range(3):
        try:
            if "nc" not in _NC_CACHE:
                _NC_CACHE["nc"] = _build_nc()
            res = run_bass_kernel_spmd(_NC_CACHE["nc"], in_maps,
                                       list(range(N_CORES)))
            core_outs = [np.asarray(res.results[c]["outw"])
                         for c in range(N_CORES)]
            break
        except Exception as e:  # transient axon/NRT faults: rebuild + retry
            last_err = e
            _NC_CACHE.clear()
            time.sleep(5 * (attempt + 1))
    else:
        raise last_err
    return _finish(core_outs, verts32, points32, na, pp32, vvb32)
